# revision 8
# baseline (speedup 1.0000x reference)
"""GCN (4-layer, PyG GCNConv-style) Trainium2 Bass kernel, SPMD over 8 NeuronCores.

Strategy
--------
Nodes are sharded round-robin-free (contiguous blocks) across 8 cores; edges are
partitioned by destination node.  Per layer:
  1. transform: each core computes h_t = dinv * (h_relu @ W) for its own rows
     (PE matmul, bf16), writes them to an HBM staging buffer.
  2. AllGather the staged rows so each core holds the full [N, 128]-padded
     bf16 feature table in HBM.
  3. gather: per-edge source rows fetched with dma_gather (256B descriptors).
  4. scatter-add: one-hot segment matrices S (fp8, host-precomputed, streamed
     from HBM) contract gathered message tiles on the TensorEngine into PSUM,
     accumulating per-destination sums; epilogue applies dinv[dst], bias, relu.
Final classifier + log_softmax computed per-core on its own rows.

All edge sorting / padding / one-hot construction happens on the host in numpy
inside kernel().  The dma_gather int16 index limit (32767) is handled by
splitting messages into two halves by source row (< 32768 / >= 32768) with a
re-based source view for the second half.
"""

import math
import os
import sys

import numpy as np

sys.path.insert(0, "/opt/trn_rl_repo")

import ml_dtypes  # noqa: E402

NCORES = 8
TILE = 128
D = 96
HALF = 32768  # int16-addressable row limit for dma_gather indices
ST_TILES = 4  # tiles per supertile (one gather call pair per supertile)
N_LAYERS = 4


def _ceil_div(a, b):
    return -(-a // b)


# ---------------------------------------------------------------------------
# Host-side preprocessing
# ---------------------------------------------------------------------------


class Plan:
    """Shared (core-independent) structure + per-core data arrays."""

    pass


def _prep(x, edge_index):
    """Build the shared chunk structure and per-core input arrays."""
    x = np.asarray(x, dtype=np.float32)
    edge_index = np.asarray(edge_index, dtype=np.int64)
    N, d_in = x.shape
    assert d_in == D
    NPC = N // NCORES
    assert NPC * NCORES == N
    NT = _ceil_div(NPC, TILE)
    NTP = NT * TILE
    NST = _ceil_div(NT, ST_TILES)

    src = edge_index[0]
    dst = edge_index[1]
    loop = np.arange(N, dtype=np.int64)
    src_all = np.concatenate([src, loop])
    dst_all = np.concatenate([dst, loop])
    M = src_all.shape[0]

    deg = np.bincount(dst_all, minlength=N).astype(np.float32)
    dinv = (1.0 / np.sqrt(deg)).astype(np.float32)

    core = dst_all // NPC
    tl = (dst_all % NPC) // TILE
    hb = (src_all >= HALF).astype(np.int64)
    gid = (core * NT + tl) * 2 + hb
    order = np.argsort(gid, kind="stable")
    gsrc = src_all[order]
    gdst = dst_all[order]
    gid_s = gid[order]

    counts = np.bincount(gid, minlength=NCORES * NT * 2).reshape(NCORES, NT, 2)
    # chunks per (tile, half): max over cores so the instruction stream is shared
    KA = _ceil_div(counts[:, :, 0].max(axis=0), TILE)  # [NT]
    KB = _ceil_div(counts[:, :, 1].max(axis=0), TILE)  # [NT]
    K = KA + KB

    # supertile structure ---------------------------------------------------
    st_tiles = [list(range(s * ST_TILES, min((s + 1) * ST_TILES, NT))) for s in range(NST)]

    # global chunk ids: per supertile: A-chunks tile-major, then B-chunks
    gbaseA = np.zeros(NT, dtype=np.int64)
    gbaseB = np.zeros(NT, dtype=np.int64)
    # position of chunk within its supertile's msg buffer
    lbaseA = np.zeros(NT, dtype=np.int64)
    lbaseB = np.zeros(NT, dtype=np.int64)
    st_of_tile = np.zeros(NT, dtype=np.int64)
    st_chunk_off = np.zeros(NST, dtype=np.int64)  # global chunk id of supertile start
    st_nchunks = np.zeros(NST, dtype=np.int64)
    g = 0
    for s, tiles in enumerate(st_tiles):
        st_chunk_off[s] = g
        off = 0
        for t in tiles:
            st_of_tile[t] = s
            gbaseA[t] = g
            lbaseA[t] = off
            g += KA[t]
            off += KA[t]
        for t in tiles:
            gbaseB[t] = g
            lbaseB[t] = off
            g += KB[t]
            off += KB[t]
        st_nchunks[s] = off
    TOTCH = g

    # gather calls: (supertile, half) -> num idxs + col offset in idx array
    call_n = np.zeros((NST, 2), dtype=np.int64)
    for s, tiles in enumerate(st_tiles):
        call_n[s, 0] = sum(KA[t] for t in tiles) * TILE
        call_n[s, 1] = sum(KB[t] for t in tiles) * TILE
    call_coloff = np.zeros((NST, 2), dtype=np.int64)
    col = 0
    for s in range(NST):
        for h in range(2):
            call_coloff[s, h] = col
            col += call_n[s, h] // 16
    TOTIDX16 = col  # idx array free-dim length (int16 columns)

    # slot of each (tile, half) group's first message within its gather call
    call_slot_base = np.zeros((NT, 2), dtype=np.int64)
    for s, tiles in enumerate(st_tiles):
        offA = 0
        offB = 0
        for t in tiles:
            call_slot_base[t, 0] = offA
            offA += KA[t] * TILE
            call_slot_base[t, 1] = offB
            offB += KB[t] * TILE

    # ---------------------------------------------------------------- per-msg
    # position within (core, tile, half) group
    gstart = np.zeros(NCORES * NT * 2 + 1, dtype=np.int64)
    np.cumsum(np.bincount(gid_s, minlength=NCORES * NT * 2), out=gstart[1:])
    pos = np.arange(M, dtype=np.int64) - gstart[gid_s]

    m_core = gid_s // (NT * 2)
    m_tile = (gid_s // 2) % NT
    m_half = gid_s % 2
    m_chunk_in_group = pos // TILE
    m_part = pos % TILE
    m_gchunk = np.where(m_half == 0, gbaseA[m_tile], gbaseB[m_tile]) + m_chunk_in_group
    m_dstloc = gdst - (m_core * NPC + m_tile * TILE)
    m_idx16 = np.where(m_half == 0, gsrc, gsrc - HALF).astype(np.int16)
    m_slot = (
        call_slot_base[m_tile, m_half] + pos
    )  # slot within the gather call
    m_col = call_coloff[st_of_tile[m_tile], m_half] + m_slot // 16
    m_row16 = m_slot % 16

    # ---------------------------------------------------------------- arrays
    plan = Plan()
    plan.N, plan.NPC, plan.NT, plan.NTP, plan.NST = N, NPC, NT, NTP, NST
    plan.st_tiles = st_tiles
    plan.KA, plan.KB, plan.K = KA, KB, K
    plan.gbaseA, plan.gbaseB = gbaseA, gbaseB
    plan.st_chunk_off, plan.st_nchunks = st_chunk_off, st_nchunks
    plan.call_n, plan.call_coloff = call_n, call_coloff
    plan.TOTCH, plan.TOTIDX16 = TOTCH, TOTIDX16
    plan.dinv = dinv

    per_core = []
    for c in range(NCORES):
        sel = m_core == c
        # S one-hot [128, TOTCH*128] fp8
        S = np.zeros((TILE, TOTCH * TILE), dtype=ml_dtypes.float8_e4m3)
        S[m_part[sel], m_gchunk[sel] * TILE + m_dstloc[sel]] = 1.0
        # idx [128, TOTIDX16] int16 (wrapped by 16, replicated across 8 groups)
        idx16 = np.zeros((16, TOTIDX16), dtype=np.int16)
        idx16[m_row16[sel], m_col[sel]] = m_idx16[sel]
        idx = np.tile(idx16, (8, 1))
        # xT [96, NTP] f32
        xT = np.zeros((D, NTP), dtype=np.float32)
        xT[:, :NPC] = x[c * NPC : (c + 1) * NPC].T
        # dinvT replicated [96, NTP]
        dinvT = np.ones((D, NTP), dtype=np.float32)
        dinvT[:, :NPC] = dinv[c * NPC : (c + 1) * NPC][None, :]
        # dinv per own row, tile-column layout [128, NT]
        downv = np.ones((TILE, NT), dtype=np.float32)
        dv = dinv[c * NPC : (c + 1) * NPC]
        dvp = np.zeros(NTP, dtype=np.float32)
        dvp[:NPC] = dv
        downv[:, :] = dvp.reshape(NT, TILE).T
        per_core.append(dict(S=S, idx=idx, xT=xT, dinvT=dinvT, dinvown=downv))
    plan.per_core = per_core
    return plan


# ---------------------------------------------------------------------------
# Bass program builder
# ---------------------------------------------------------------------------


def _build(plan, repeats=1):
    import concourse.bass as bass
    import concourse.bacc as bacc
    import concourse.mybir as mybir
    import concourse.tile as tile

    f32 = mybir.dt.float32
    bf16 = mybir.dt.bfloat16
    fp8 = mybir.dt.float8e4
    i16 = mybir.dt.int16
    AF = mybir.ActivationFunctionType
    ALU = mybir.AluOpType

    N, NPC, NT, NTP, NST = plan.N, plan.NPC, plan.NT, plan.NTP, plan.NST
    TOTCH, TOTIDX16 = plan.TOTCH, plan.TOTIDX16
    KA, KB = plan.KA, plan.KB

    nc = bacc.Bacc(None, target_bir_lowering=False)

    xT_p = nc.declare_dram_parameter("xT", [D, NTP], f32, isOutput=False)
    idx_p = nc.declare_dram_parameter("idx", [TILE, TOTIDX16], i16, isOutput=False)
    s_p = nc.declare_dram_parameter("S", [TILE, TOTCH * TILE], fp8, isOutput=False)
    dinvT_p = nc.declare_dram_parameter("dinvT", [D, NTP], f32, isOutput=False)
    dinvown_p = nc.declare_dram_parameter("dinvown", [TILE, NT], f32, isOutput=False)
    biasT_p = nc.declare_dram_parameter("biasT", [D, N_LAYERS], f32, isOutput=False)
    brep_p = nc.declare_dram_parameter("brep", [TILE, 4], f32, isOutput=False)
    w_p = nc.declare_dram_parameter("W", [D, N_LAYERS * D], f32, isOutput=False)
    wl_p = nc.declare_dram_parameter("Wl", [D, 4], f32, isOutput=False)
    out_p = nc.declare_dram_parameter("out", [NPC, 4], f32, isOutput=True)

    replica_groups = [list(range(NCORES))]

    with tile.TileContext(nc) as tc:
        with (
            tc.tile_pool(name="persist", bufs=1) as persist,
            tc.tile_pool(name="hrelu", bufs=2) as hrelu_pool,
            tc.tile_pool(name="msg", bufs=2) as msg_pool,
            tc.tile_pool(name="spool", bufs=2) as s_pool,
            tc.tile_pool(name="tmp", bufs=3) as tmp_pool,
            tc.tile_pool(name="stage", bufs=3) as stage_pool,
            tc.tile_pool(name="small", bufs=3) as small_pool,
            tc.tile_pool(name="psA", bufs=6, space="PSUM") as psA_pool,
            tc.tile_pool(name="ps3", bufs=2, space="PSUM") as ps3_pool,
            tc.tile_pool(name="dram", bufs=2, space="DRAM") as dram_pool,
        ):
            # ------------------------------------------------- persistent loads
            xT_sb = persist.tile([D, NTP], bf16, tag="xT")
            nc.gpsimd.dma_start(xT_sb[:], xT_p[:])  # f32 -> bf16 cast DMA
            idx_sb = persist.tile([TILE, TOTIDX16], i16, tag="idx")
            nc.sync.dma_start(idx_sb[:], idx_p[:])
            dinvT_sb = persist.tile([D, NTP], f32, tag="dinvT")
            nc.sync.dma_start(dinvT_sb[:], dinvT_p[:])
            dinvown_sb = persist.tile([TILE, NT], f32, tag="dinvown")
            nc.sync.dma_start(dinvown_sb[:], dinvown_p[:])
            biasT_sb = persist.tile([D, N_LAYERS], f32, tag="biasT")
            nc.sync.dma_start(biasT_sb[:], biasT_p[:])
            brep_sb = persist.tile([TILE, 4], f32, tag="brep")
            nc.sync.dma_start(brep_sb[:], brep_p[:])
            w_sb = persist.tile([D, N_LAYERS * D], bf16, tag="W")
            nc.gpsimd.dma_start(w_sb[:], w_p[:])
            wl_sb = persist.tile([D, 4], bf16, tag="Wl")
            nc.gpsimd.dma_start(wl_sb[:], wl_p[:])

            cur_T = xT_sb  # [96, NTP] bf16, transposed feature table

            for rep in range(repeats):
              cur_T = xT_sb
              for layer in range(N_LAYERS):
                # ---------------------------------------- transform + stage out
                ag_in = dram_pool.tile([NPC, TILE], bf16, tag="ag_in")
                ag_out = dram_pool.tile([N, TILE], bf16, tag="ag_out", addr_space="Shared")
                for t in range(NT):
                    w = min(TILE, NPC - t * TILE)
                    ps3 = ps3_pool.tile([TILE, D], f32, tag="ps3")
                    nc.tensor.matmul(
                        ps3[:w],
                        cur_T[:, t * TILE : t * TILE + w],
                        w_sb[:, layer * D : (layer + 1) * D],
                        start=True,
                        stop=True,
                    )
                    st = stage_pool.tile([TILE, TILE], bf16, tag="stage")
                    nc.vector.memset(st[:w, D:TILE], 0.0)
                    nc.scalar.activation(
                        st[:w, 0:D], ps3[:w], AF.Copy, scale=dinvown_sb[:w, t : t + 1]
                    )
                    nc.sync.dma_start(ag_in[t * TILE : t * TILE + w, :], st[:w])

                nc.gpsimd.collective_compute(
                    "AllGather",
                    ALU.bypass,
                    replica_groups=replica_groups,
                    ins=[ag_in[:]],
                    outs=[ag_out[:]],
                )

                # ---------------------------------------- gather + aggregate
                new_T = hrelu_pool.tile([D, NTP], bf16, tag="hrelu")
                for s, tiles in enumerate(plan.st_tiles):
                    nch = int(plan.st_nchunks[s])
                    goff = int(plan.st_chunk_off[s])
                    msg = msg_pool.tile([TILE, nch * TILE], bf16, tag="msg")
                    msg3 = msg[:].rearrange("p (c e) -> p c e", e=TILE)
                    s_sb = s_pool.tile([TILE, nch * TILE], fp8, tag="spool")
                    nc.sync.dma_start(
                        s_sb[:], s_p[:, goff * TILE : (goff + nch) * TILE]
                    )
                    # gather calls (A half then B half), split to <=1024
                    # indices per call (SWDGE descriptor-ring capacity)
                    GMAX = 1024
                    ch_off = 0
                    for h in range(2):
                        n = int(plan.call_n[s, h])
                        if n == 0:
                            continue
                        coloff = int(plan.call_coloff[s, h])
                        in_ap = ag_out[:] if h == 0 else ag_out[HALF:N, :]
                        for c0 in range(0, n, GMAX):
                            nn = min(GMAX, n - c0)
                            out_ap = msg3[
                                :, ch_off + c0 // TILE : ch_off + (c0 + nn) // TILE, :
                            ]
                            nc.gpsimd.dma_gather(
                                out_ap,
                                in_ap,
                                idx_sb[:, coloff + c0 // 16 : coloff + (c0 + nn) // 16],
                                num_idxs=nn,
                                num_idxs_reg=nn,
                                elem_size=TILE,
                            )
                        ch_off += n // TILE
                    # per-tile accumulation, tile-major chunk order
                    for t in tiles:
                        kA, kB = int(KA[t]), int(KB[t])
                        nchunks_t = kA + kB
                        assert nchunks_t > 0
                        lA = int(plan.gbaseA[t] - goff)
                        lB = int(plan.gbaseB[t] - goff)
                        locs = [lA + j for j in range(kA)] + [lB + j for j in range(kB)]
                        psA = psA_pool.tile([D, TILE], f32, tag="psA")
                        for ji, j in enumerate(locs):
                            nc.tensor.matmul(
                                psA[:],
                                msg3[:, j, 0:D],
                                s_sb[:, j * TILE : (j + 1) * TILE],
                                start=(ji == 0),
                                stop=(ji == nchunks_t - 1),
                            )
                        tmp = tmp_pool.tile([D, TILE], f32, tag="tmp")
                        nc.vector.tensor_tensor(
                            tmp[:], psA[:], dinvT_sb[:, t * TILE : (t + 1) * TILE], ALU.mult
                        )
                        nc.scalar.activation(
                            new_T[:, t * TILE : (t + 1) * TILE],
                            tmp[:],
                            AF.Relu,
                            bias=biasT_sb[:, layer : layer + 1],
                        )
                cur_T = new_T

            # ------------------------------------------------- classifier
            for t in range(NT):
                w = min(TILE, NPC - t * TILE)
                psf = ps3_pool.tile([TILE, D], f32, tag="ps3")
                nc.tensor.matmul(
                    psf[:w, 0:4],
                    cur_T[:, t * TILE : t * TILE + w],
                    wl_sb[:],
                    start=True,
                    stop=True,
                )
                xb = small_pool.tile([TILE, 4], f32, tag="xb")
                nc.vector.tensor_tensor(xb[:w], psf[:w, 0:4], brep_sb[:w], ALU.add)
                negm = small_pool.tile([TILE, 1], f32, tag="negm")
                nc.vector.tensor_reduce(
                    negm[:w], xb[:w], mybir.AxisListType.X, ALU.max, negate=True
                )
                ex = small_pool.tile([TILE, 4], f32, tag="ex")
                sumexp = small_pool.tile([TILE, 1], f32, tag="sumexp")
                nc.scalar.activation(
                    ex[:w], xb[:w], AF.Exp, bias=negm[:w], accum_out=sumexp[:w]
                )
                lse = small_pool.tile([TILE, 1], f32, tag="lse")
                nc.scalar.activation(lse[:w], sumexp[:w], AF.Ln)
                shift = small_pool.tile([TILE, 1], f32, tag="shift")
                nc.vector.tensor_sub(shift[:w], negm[:w], lse[:w])
                outt = small_pool.tile([TILE, 4], f32, tag="outt")
                nc.vector.tensor_scalar_add(outt[:w], xb[:w], shift[:w])
                nc.sync.dma_start(out_p[t * TILE : t * TILE + w, :], outt[:w])

    nc.compile()
    return nc


# ---------------------------------------------------------------------------
# in_maps assembly
# ---------------------------------------------------------------------------


def _in_maps(plan, W0, b0, W1, b1, W2, b2, W3, b3, Wl, bl):
    Ws = np.concatenate(
        [np.asarray(w, np.float32) for w in (W0, W1, W2, W3)], axis=1
    )  # [96, 4*96]
    biasT = np.stack(
        [np.asarray(b, np.float32) for b in (b0, b1, b2, b3)], axis=1
    )  # [96, 4]
    brep = np.tile(np.asarray(bl, np.float32)[None, :], (TILE, 1))  # [128, 4]
    wl = np.asarray(Wl, np.float32)
    maps = []
    for c in range(NCORES):
        pc = plan.per_core[c]
        maps.append(
            {
                "xT": pc["xT"],
                "idx": pc["idx"],
                "S": pc["S"],
                "dinvT": pc["dinvT"],
                "dinvown": pc["dinvown"],
                "biasT": biasT,
                "brep": brep,
                "W": Ws,
                "Wl": wl,
                "out": np.zeros((plan.NPC, 4), np.float32),
            }
        )
    return maps


# ---------------------------------------------------------------------------
# public entry point
# ---------------------------------------------------------------------------

_CACHE = {}


def _get_compiled(plan):
    return _build(plan)


def kernel(x, edge_index, W0, b0, W1, b1, W2, b2, W3, b3, Wl, bl):
    from concourse.bass_utils import run_bass_kernel_spmd

    x = np.asarray(x, np.float32)
    edge_index = np.asarray(edge_index, np.int64)
    plan = _prep(x, edge_index)
    nc = _get_compiled(plan)
    in_maps = _in_maps(plan, W0, b0, W1, b1, W2, b2, W3, b3, Wl, bl)
    res = run_bass_kernel_spmd(nc, in_maps, core_ids=list(range(NCORES)))
    out = np.concatenate([res.results[c]["out"] for c in range(NCORES)], axis=0)
    return out.astype(np.float32)


# revision 15
# speedup vs baseline: 1.6257x; 1.6257x over previous
"""GCN (4-layer, PyG GCNConv-style) Trainium2 Bass kernel, SPMD over 8 NeuronCores.

Strategy
--------
Nodes are sharded round-robin-free (contiguous blocks) across 8 cores; edges are
partitioned by destination node.  Per layer:
  1. transform: each core computes h_t = dinv * (h_relu @ W) for its own rows
     (PE matmul, bf16), writes them to an HBM staging buffer.
  2. AllGather the staged rows so each core holds the full [N, 128]-padded
     bf16 feature table in HBM.
  3. gather: per-edge source rows fetched with dma_gather (256B descriptors).
  4. scatter-add: one-hot segment matrices S (fp8, host-precomputed, streamed
     from HBM) contract gathered message tiles on the TensorEngine into PSUM,
     accumulating per-destination sums; epilogue applies dinv[dst], bias, relu.
Final classifier + log_softmax computed per-core on its own rows.

All edge sorting / padding / one-hot construction happens on the host in numpy
inside kernel().  The dma_gather int16 index limit (32767) is handled by
splitting messages into two halves by source row (< 32768 / >= 32768) with a
re-based source view for the second half.
"""

import math
import os
import sys

import numpy as np

sys.path.insert(0, "/opt/trn_rl_repo")

import ml_dtypes  # noqa: E402

NCORES = 8
TILE = 128
D = 96
HALF = 32768  # int16-addressable row limit for dma_gather indices
ST_TILES = 4  # tiles per supertile (one gather call pair per supertile)
N_LAYERS = 4
SINGLE_PACKET = False


def _ceil_div(a, b):
    return -(-a // b)


# ---------------------------------------------------------------------------
# Host-side preprocessing
# ---------------------------------------------------------------------------


class Plan:
    """Shared (core-independent) structure + per-core data arrays."""

    pass


def _prep(x, edge_index):
    """Build the shared chunk structure and per-core input arrays."""
    x = np.asarray(x, dtype=np.float32)
    edge_index = np.asarray(edge_index, dtype=np.int64)
    N, d_in = x.shape
    assert d_in == D
    NPC = N // NCORES
    assert NPC * NCORES == N
    NT = _ceil_div(NPC, TILE)
    NTP = NT * TILE
    NST = _ceil_div(NT, ST_TILES)

    src = edge_index[0]
    dst = edge_index[1]
    loop = np.arange(N, dtype=np.int64)
    src_all = np.concatenate([src, loop])
    dst_all = np.concatenate([dst, loop])
    M = src_all.shape[0]

    deg = np.bincount(dst_all, minlength=N).astype(np.float32)
    dinv = (1.0 / np.sqrt(deg)).astype(np.float32)

    core = dst_all // NPC
    tl = (dst_all % NPC) // TILE
    hb = (src_all >= HALF).astype(np.int64)
    gid = (core * NT + tl) * 2 + hb
    order = np.argsort(gid, kind="stable")
    gsrc = src_all[order]
    gdst = dst_all[order]
    gid_s = gid[order]

    counts = np.bincount(gid, minlength=NCORES * NT * 2).reshape(NCORES, NT, 2)
    # chunks per (tile, half): max over cores so the instruction stream is shared
    KA = _ceil_div(counts[:, :, 0].max(axis=0), TILE)  # [NT]
    KB = _ceil_div(counts[:, :, 1].max(axis=0), TILE)  # [NT]
    K = KA + KB

    # supertile structure ---------------------------------------------------
    st_tiles = [list(range(s * ST_TILES, min((s + 1) * ST_TILES, NT))) for s in range(NST)]

    # global chunk ids: per supertile: A-chunks tile-major, then B-chunks
    gbaseA = np.zeros(NT, dtype=np.int64)
    gbaseB = np.zeros(NT, dtype=np.int64)
    # position of chunk within its supertile's msg buffer
    lbaseA = np.zeros(NT, dtype=np.int64)
    lbaseB = np.zeros(NT, dtype=np.int64)
    st_of_tile = np.zeros(NT, dtype=np.int64)
    st_chunk_off = np.zeros(NST, dtype=np.int64)  # global chunk id of supertile start
    st_nchunks = np.zeros(NST, dtype=np.int64)
    g = 0
    for s, tiles in enumerate(st_tiles):
        st_chunk_off[s] = g
        off = 0
        for t in tiles:
            st_of_tile[t] = s
            gbaseA[t] = g
            lbaseA[t] = off
            g += KA[t]
            off += KA[t]
        for t in tiles:
            gbaseB[t] = g
            lbaseB[t] = off
            g += KB[t]
            off += KB[t]
        st_nchunks[s] = off
    TOTCH = g

    # gather calls: (supertile, half) -> num idxs + col offset in idx array
    call_n = np.zeros((NST, 2), dtype=np.int64)
    for s, tiles in enumerate(st_tiles):
        call_n[s, 0] = sum(KA[t] for t in tiles) * TILE
        call_n[s, 1] = sum(KB[t] for t in tiles) * TILE
    call_coloff = np.zeros((NST, 2), dtype=np.int64)
    col = 0
    for s in range(NST):
        for h in range(2):
            call_coloff[s, h] = col
            col += call_n[s, h] // 16
    TOTIDX16 = col  # idx array free-dim length (int16 columns)

    # slot of each (tile, half) group's first message within its gather call
    call_slot_base = np.zeros((NT, 2), dtype=np.int64)
    for s, tiles in enumerate(st_tiles):
        offA = 0
        offB = 0
        for t in tiles:
            call_slot_base[t, 0] = offA
            offA += KA[t] * TILE
            call_slot_base[t, 1] = offB
            offB += KB[t] * TILE

    # ---------------------------------------------------------------- per-msg
    # position within (core, tile, half) group
    gstart = np.zeros(NCORES * NT * 2 + 1, dtype=np.int64)
    np.cumsum(np.bincount(gid_s, minlength=NCORES * NT * 2), out=gstart[1:])
    pos = np.arange(M, dtype=np.int64) - gstart[gid_s]

    m_core = gid_s // (NT * 2)
    m_tile = (gid_s // 2) % NT
    m_half = gid_s % 2
    m_chunk_in_group = pos // TILE
    m_part = pos % TILE
    m_gchunk = np.where(m_half == 0, gbaseA[m_tile], gbaseB[m_tile]) + m_chunk_in_group
    m_dstloc = gdst - (m_core * NPC + m_tile * TILE)
    m_idx16 = np.where(m_half == 0, gsrc, gsrc - HALF).astype(np.int16)
    m_slot = (
        call_slot_base[m_tile, m_half] + pos
    )  # slot within the gather call
    m_col = call_coloff[st_of_tile[m_tile], m_half] + m_slot // 16
    m_row16 = m_slot % 16

    # ---------------------------------------------------------------- arrays
    plan = Plan()
    plan.N, plan.NPC, plan.NT, plan.NTP, plan.NST = N, NPC, NT, NTP, NST
    plan.st_tiles = st_tiles
    plan.KA, plan.KB, plan.K = KA, KB, K
    plan.gbaseA, plan.gbaseB = gbaseA, gbaseB
    plan.st_chunk_off, plan.st_nchunks = st_chunk_off, st_nchunks
    plan.call_n, plan.call_coloff = call_n, call_coloff
    plan.TOTCH, plan.TOTIDX16 = TOTCH, TOTIDX16
    plan.dinv = dinv

    per_core = []
    for c in range(NCORES):
        sel = m_core == c
        # S one-hot [128, TOTCH*128] fp8
        S = np.zeros((TILE, TOTCH * TILE), dtype=ml_dtypes.float8_e4m3)
        S[m_part[sel], m_gchunk[sel] * TILE + m_dstloc[sel]] = 1.0
        # idx [128, TOTIDX16] int16 (wrapped by 16, replicated across 8 groups)
        idx16 = np.zeros((16, TOTIDX16), dtype=np.int16)
        idx16[m_row16[sel], m_col[sel]] = m_idx16[sel]
        idx = np.tile(idx16, (8, 1))
        # xT [96, NTP] f32
        xT = np.zeros((D, NTP), dtype=np.float32)
        xT[:, :NPC] = x[c * NPC : (c + 1) * NPC].T
        # dinvT replicated [96, NTP]
        dinvT = np.ones((D, NTP), dtype=np.float32)
        dinvT[:, :NPC] = dinv[c * NPC : (c + 1) * NPC][None, :]
        # dinv per own row, tile-column layout [128, NT]
        downv = np.ones((TILE, NT), dtype=np.float32)
        dv = dinv[c * NPC : (c + 1) * NPC]
        dvp = np.zeros(NTP, dtype=np.float32)
        dvp[:NPC] = dv
        downv[:, :] = dvp.reshape(NT, TILE).T
        per_core.append(dict(S=S, idx=idx, xT=xT, dinvT=dinvT, dinvown=downv))
    plan.per_core = per_core
    return plan


# ---------------------------------------------------------------------------
# Bass program builder
# ---------------------------------------------------------------------------


def _build(plan, repeats=1, skip=frozenset()):
    import concourse.bass as bass
    import concourse.bacc as bacc
    import concourse.mybir as mybir
    import concourse.tile as tile

    f32 = mybir.dt.float32
    bf16 = mybir.dt.bfloat16
    fp8 = mybir.dt.float8e4
    i16 = mybir.dt.int16
    AF = mybir.ActivationFunctionType
    ALU = mybir.AluOpType

    N, NPC, NT, NTP, NST = plan.N, plan.NPC, plan.NT, plan.NTP, plan.NST
    TOTCH, TOTIDX16 = plan.TOTCH, plan.TOTIDX16
    KA, KB = plan.KA, plan.KB

    nc = bacc.Bacc(None, target_bir_lowering=False, num_swdge_queues=4)

    xT_p = nc.declare_dram_parameter("xT", [D, NTP], f32, isOutput=False)
    idx_p = nc.declare_dram_parameter("idx", [TILE, TOTIDX16], i16, isOutput=False)
    s_p = nc.declare_dram_parameter("S", [TILE, TOTCH * TILE], fp8, isOutput=False)
    dinvT_p = nc.declare_dram_parameter("dinvT", [D, NTP], f32, isOutput=False)
    dinvown_p = nc.declare_dram_parameter("dinvown", [TILE, NT], f32, isOutput=False)
    biasT_p = nc.declare_dram_parameter("biasT", [D, N_LAYERS], f32, isOutput=False)
    brep_p = nc.declare_dram_parameter("brep", [TILE, 4], f32, isOutput=False)
    w_p = nc.declare_dram_parameter("W", [D, N_LAYERS * D], f32, isOutput=False)
    wl_p = nc.declare_dram_parameter("Wl", [D, 4], f32, isOutput=False)
    out_p = nc.declare_dram_parameter("out", [NPC, 4], f32, isOutput=True)

    replica_groups = [list(range(NCORES))]

    with tile.TileContext(nc) as tc:
        with (
            tc.tile_pool(name="persist", bufs=1) as persist,
            tc.tile_pool(name="hrelu", bufs=2) as hrelu_pool,
            tc.tile_pool(name="msg", bufs=2) as msg_pool,
            tc.tile_pool(name="spool", bufs=2) as s_pool,
            tc.tile_pool(name="tmp", bufs=3) as tmp_pool,
            tc.tile_pool(name="stage", bufs=3) as stage_pool,
            tc.tile_pool(name="small", bufs=3) as small_pool,
            tc.tile_pool(name="psA", bufs=6, space="PSUM") as psA_pool,
            tc.tile_pool(name="ps3", bufs=2, space="PSUM") as ps3_pool,
            tc.tile_pool(name="dram", bufs=2, space="DRAM") as dram_pool,
        ):
            # ------------------------------------------------- persistent loads
            xT_sb = persist.tile([D, NTP], bf16, tag="xT")
            nc.gpsimd.dma_start(xT_sb[:], xT_p[:])  # f32 -> bf16 cast DMA
            idx_sb = persist.tile([TILE, TOTIDX16], i16, tag="idx")
            nc.sync.dma_start(idx_sb[:], idx_p[:])
            dinvT_sb = persist.tile([D, NTP], f32, tag="dinvT")
            nc.sync.dma_start(dinvT_sb[:], dinvT_p[:])
            dinvown_sb = persist.tile([TILE, NT], f32, tag="dinvown")
            nc.sync.dma_start(dinvown_sb[:], dinvown_p[:])
            biasT_sb = persist.tile([D, N_LAYERS], f32, tag="biasT")
            nc.sync.dma_start(biasT_sb[:], biasT_p[:])
            brep_sb = persist.tile([TILE, 4], f32, tag="brep")
            nc.sync.dma_start(brep_sb[:], brep_p[:])
            w_sb = persist.tile([D, N_LAYERS * D], bf16, tag="W")
            nc.gpsimd.dma_start(w_sb[:], w_p[:])
            wl_sb = persist.tile([D, 4], bf16, tag="Wl")
            nc.gpsimd.dma_start(wl_sb[:], wl_p[:])

            cur_T = xT_sb  # [96, NTP] bf16, transposed feature table

            for rep in range(repeats):
              cur_T = xT_sb
              for layer in range(N_LAYERS):
                # ---------------------------------------- transform + stage out
                ag_in = dram_pool.tile([NPC, TILE], bf16, tag="ag_in")
                ag_out = dram_pool.tile(
                    [N, TILE],
                    bf16,
                    tag="ag_out",
                    addr_space="Local" if "ag" in skip else "Shared",
                )
                for t in range(NT):
                    w = min(TILE, NPC - t * TILE)
                    ps3 = ps3_pool.tile([TILE, D], f32, tag="ps3")
                    nc.tensor.matmul(
                        ps3[:w],
                        cur_T[:, t * TILE : t * TILE + w],
                        w_sb[:, layer * D : (layer + 1) * D],
                        start=True,
                        stop=True,
                    )
                    st = stage_pool.tile([TILE, TILE], bf16, tag="stage")
                    nc.vector.memset(st[:w, D:TILE], 0.0)
                    nc.scalar.activation(
                        st[:w, 0:D], ps3[:w], AF.Copy, scale=dinvown_sb[:w, t : t + 1]
                    )
                    nc.sync.dma_start(ag_in[t * TILE : t * TILE + w, :], st[:w])

                if "ag" in skip:
                    for r in range(NCORES):
                        nc.sync.dma_start(ag_out[r * NPC : (r + 1) * NPC, :], ag_in[:])
                else:
                    nc.gpsimd.collective_compute(
                        "AllGather",
                        ALU.bypass,
                        replica_groups=replica_groups,
                        ins=[ag_in[:]],
                        outs=[ag_out[:]],
                    )

                # ---------------------------------------- gather + aggregate
                new_T = hrelu_pool.tile([D, NTP], bf16, tag="hrelu")
                for s, tiles in enumerate(plan.st_tiles):
                    nch = int(plan.st_nchunks[s])
                    goff = int(plan.st_chunk_off[s])
                    msg = msg_pool.tile([TILE, nch * TILE], bf16, tag="msg")
                    msg3 = msg[:].rearrange("p (c e) -> p c e", e=TILE)
                    s_sb = s_pool.tile([TILE, nch * TILE], fp8, tag="spool")
                    g_live = "gather" not in skip
                    s_live = "sdma" not in skip
                    if s_live:
                        nc.sync.dma_start(
                            s_sb[:], s_p[:, goff * TILE : (goff + nch) * TILE]
                        )
                    else:
                        nc.sync.dma_start(
                            s_sb[:, 0:TILE], s_p[:, goff * TILE : (goff + 1) * TILE]
                        )
                    # gather calls (A half then B half), split to <=1024
                    # indices per call (SWDGE descriptor-ring capacity)
                    GMAX = 1024
                    ch_off = 0
                    gq = [0]
                    for h in range(2):
                        n = int(plan.call_n[s, h])
                        if n == 0:
                            continue
                        coloff = int(plan.call_coloff[s, h])
                        in_ap = ag_out[:] if h == 0 else ag_out[HALF:N, :]
                        if not g_live:
                            if h == 0:
                                nc.gpsimd.dma_gather(
                                    msg3[:, 0:1, :], in_ap,
                                    idx_sb[:, coloff : coloff + 8],
                                    num_idxs=TILE, num_idxs_reg=TILE, elem_size=TILE,
                                )
                            continue
                        for c0 in range(0, n, GMAX):
                            nn = min(GMAX, n - c0)
                            out_ap = msg3[
                                :, ch_off + c0 // TILE : ch_off + (c0 + nn) // TILE, :
                            ]
                            nc.gpsimd.dma_gather(
                                out_ap,
                                in_ap,
                                idx_sb[:, coloff + c0 // 16 : coloff + (c0 + nn) // 16],
                                num_idxs=nn,
                                num_idxs_reg=nn,
                                elem_size=TILE,
                                queue_num=gq[0] % 4,
                                single_packet=SINGLE_PACKET,
                            )
                            gq[0] += 1
                        ch_off += n // TILE
                    # per-tile accumulation, tile-major chunk order
                    for t in tiles:
                        kA, kB = int(KA[t]), int(KB[t])
                        nchunks_t = kA + kB
                        assert nchunks_t > 0
                        lA = int(plan.gbaseA[t] - goff)
                        lB = int(plan.gbaseB[t] - goff)
                        locs = [lA + j for j in range(kA)] + [lB + j for j in range(kB)]
                        psA = psA_pool.tile([D, TILE], f32, tag="psA")
                        for ji, j in enumerate(locs if "mm" not in skip else locs[:1]):
                            nc.tensor.matmul(
                                psA[:],
                                msg3[:, j if g_live else 0, 0:D],
                                s_sb[:, (j if s_live else 0) * TILE : ((j if s_live else 0) + 1) * TILE],
                                start=(ji == 0),
                                stop=(ji == nchunks_t - 1),
                            )
                        tmp = tmp_pool.tile([D, TILE], f32, tag="tmp")
                        nc.vector.tensor_tensor(
                            tmp[:], psA[:], dinvT_sb[:, t * TILE : (t + 1) * TILE], ALU.mult
                        )
                        nc.scalar.activation(
                            new_T[:, t * TILE : (t + 1) * TILE],
                            tmp[:],
                            AF.Relu,
                            bias=biasT_sb[:, layer : layer + 1],
                        )
                cur_T = new_T

            # ------------------------------------------------- classifier
            for t in range(NT):
                w = min(TILE, NPC - t * TILE)
                psf = ps3_pool.tile([TILE, D], f32, tag="ps3")
                nc.tensor.matmul(
                    psf[:w, 0:4],
                    cur_T[:, t * TILE : t * TILE + w],
                    wl_sb[:],
                    start=True,
                    stop=True,
                )
                xb = small_pool.tile([TILE, 4], f32, tag="xb")
                nc.vector.tensor_tensor(xb[:w], psf[:w, 0:4], brep_sb[:w], ALU.add)
                negm = small_pool.tile([TILE, 1], f32, tag="negm")
                nc.vector.tensor_reduce(
                    negm[:w], xb[:w], mybir.AxisListType.X, ALU.max, negate=True
                )
                ex = small_pool.tile([TILE, 4], f32, tag="ex")
                sumexp = small_pool.tile([TILE, 1], f32, tag="sumexp")
                nc.scalar.activation(
                    ex[:w], xb[:w], AF.Exp, bias=negm[:w], accum_out=sumexp[:w]
                )
                lse = small_pool.tile([TILE, 1], f32, tag="lse")
                nc.scalar.activation(lse[:w], sumexp[:w], AF.Ln)
                shift = small_pool.tile([TILE, 1], f32, tag="shift")
                nc.vector.tensor_sub(shift[:w], negm[:w], lse[:w])
                outt = small_pool.tile([TILE, 4], f32, tag="outt")
                nc.vector.tensor_scalar_add(outt[:w], xb[:w], shift[:w])
                nc.sync.dma_start(out_p[t * TILE : t * TILE + w, :], outt[:w])

    nc.compile()
    return nc


# ---------------------------------------------------------------------------
# in_maps assembly
# ---------------------------------------------------------------------------


def _in_maps(plan, W0, b0, W1, b1, W2, b2, W3, b3, Wl, bl):
    Ws = np.concatenate(
        [np.asarray(w, np.float32) for w in (W0, W1, W2, W3)], axis=1
    )  # [96, 4*96]
    biasT = np.stack(
        [np.asarray(b, np.float32) for b in (b0, b1, b2, b3)], axis=1
    )  # [96, 4]
    brep = np.tile(np.asarray(bl, np.float32)[None, :], (TILE, 1))  # [128, 4]
    wl = np.asarray(Wl, np.float32)
    maps = []
    for c in range(NCORES):
        pc = plan.per_core[c]
        maps.append(
            {
                "xT": pc["xT"],
                "idx": pc["idx"],
                "S": pc["S"],
                "dinvT": pc["dinvT"],
                "dinvown": pc["dinvown"],
                "biasT": biasT,
                "brep": brep,
                "W": Ws,
                "Wl": wl,
                "out": np.zeros((plan.NPC, 4), np.float32),
            }
        )
    return maps


# ---------------------------------------------------------------------------
# public entry point
# ---------------------------------------------------------------------------

_CACHE = {}


def _get_compiled(plan):
    return _build(plan)


def kernel(x, edge_index, W0, b0, W1, b1, W2, b2, W3, b3, Wl, bl):
    from concourse.bass_utils import run_bass_kernel_spmd

    x = np.asarray(x, np.float32)
    edge_index = np.asarray(edge_index, np.int64)
    plan = _prep(x, edge_index)
    nc = _get_compiled(plan)
    in_maps = _in_maps(plan, W0, b0, W1, b1, W2, b2, W3, b3, Wl, bl)
    res = run_bass_kernel_spmd(nc, in_maps, core_ids=list(range(NCORES)))
    out = np.concatenate([res.results[c]["out"] for c in range(NCORES)], axis=0)
    return out.astype(np.float32)


# revision 17
# speedup vs baseline: 1.6947x; 1.0425x over previous
"""GCN (4-layer, PyG GCNConv-style) Trainium2 Bass kernel, SPMD over 8 NeuronCores.

Strategy
--------
Nodes are sharded round-robin-free (contiguous blocks) across 8 cores; edges are
partitioned by destination node.  Per layer:
  1. transform: each core computes h_t = dinv * (h_relu @ W) for its own rows
     (PE matmul, bf16), writes them to an HBM staging buffer.
  2. AllGather the staged rows so each core holds the full [N, 128]-padded
     bf16 feature table in HBM.
  3. gather: per-edge source rows fetched with dma_gather (256B descriptors).
  4. scatter-add: one-hot segment matrices S (fp8, host-precomputed, streamed
     from HBM) contract gathered message tiles on the TensorEngine into PSUM,
     accumulating per-destination sums; epilogue applies dinv[dst], bias, relu.
Final classifier + log_softmax computed per-core on its own rows.

All edge sorting / padding / one-hot construction happens on the host in numpy
inside kernel().  The dma_gather int16 index limit (32767) is handled by
splitting messages into two halves by source row (< 32768 / >= 32768) with a
re-based source view for the second half.

Perf-critical settings (measured on axon-tunneled TRN2):
  * num_swdge_queues=4 with gather calls round-robined over queue 0..3 —
    SWDGE descriptor *generation* on the Q7 is the gather bottleneck
    (~11.3 ns/desc on one queue); 4 queues generate concurrently (~4x).
  * single_packet=False on dma_gather — ~2x faster end-to-end.
  * dma_gather calls capped at 1024 indices: the per-queue descriptor ring
    holds 1024 descriptors; larger calls wedge or kill the device
    (NRT_EXEC_UNIT_UNRECOVERABLE), independent of dynamic_dma_scratch_size.
Measured ~2.2-2.8 ms per forward pass (repeat-delta timing), rel err 6e-4.
"""

import math
import os
import sys

import numpy as np

sys.path.insert(0, "/opt/trn_rl_repo")

import ml_dtypes  # noqa: E402

NCORES = 8
TILE = 128
D = 96
HALF = 32768  # int16-addressable row limit for dma_gather indices
ST_TILES = 4  # tiles per supertile (one gather call pair per supertile)
N_LAYERS = 4
SINGLE_PACKET = False
MSG_BUFS = 2


def _ceil_div(a, b):
    return -(-a // b)


# ---------------------------------------------------------------------------
# Host-side preprocessing
# ---------------------------------------------------------------------------


class Plan:
    """Shared (core-independent) structure + per-core data arrays."""

    pass


def _prep(x, edge_index):
    """Build the shared chunk structure and per-core input arrays."""
    x = np.asarray(x, dtype=np.float32)
    edge_index = np.asarray(edge_index, dtype=np.int64)
    N, d_in = x.shape
    assert d_in == D
    NPC = N // NCORES
    assert NPC * NCORES == N
    NT = _ceil_div(NPC, TILE)
    NTP = NT * TILE
    NST = _ceil_div(NT, ST_TILES)

    src = edge_index[0]
    dst = edge_index[1]
    loop = np.arange(N, dtype=np.int64)
    src_all = np.concatenate([src, loop])
    dst_all = np.concatenate([dst, loop])
    M = src_all.shape[0]

    deg = np.bincount(dst_all, minlength=N).astype(np.float32)
    dinv = (1.0 / np.sqrt(deg)).astype(np.float32)

    core = dst_all // NPC
    tl = (dst_all % NPC) // TILE
    hb = (src_all >= HALF).astype(np.int64)
    gid = (core * NT + tl) * 2 + hb
    order = np.argsort(gid, kind="stable")
    gsrc = src_all[order]
    gdst = dst_all[order]
    gid_s = gid[order]

    counts = np.bincount(gid, minlength=NCORES * NT * 2).reshape(NCORES, NT, 2)
    # chunks per (tile, half): max over cores so the instruction stream is shared
    KA = _ceil_div(counts[:, :, 0].max(axis=0), TILE)  # [NT]
    KB = _ceil_div(counts[:, :, 1].max(axis=0), TILE)  # [NT]
    K = KA + KB

    # supertile structure ---------------------------------------------------
    st_tiles = [list(range(s * ST_TILES, min((s + 1) * ST_TILES, NT))) for s in range(NST)]

    # global chunk ids: per supertile: A-chunks tile-major, then B-chunks
    gbaseA = np.zeros(NT, dtype=np.int64)
    gbaseB = np.zeros(NT, dtype=np.int64)
    # position of chunk within its supertile's msg buffer
    lbaseA = np.zeros(NT, dtype=np.int64)
    lbaseB = np.zeros(NT, dtype=np.int64)
    st_of_tile = np.zeros(NT, dtype=np.int64)
    st_chunk_off = np.zeros(NST, dtype=np.int64)  # global chunk id of supertile start
    st_nchunks = np.zeros(NST, dtype=np.int64)
    g = 0
    for s, tiles in enumerate(st_tiles):
        st_chunk_off[s] = g
        off = 0
        for t in tiles:
            st_of_tile[t] = s
            gbaseA[t] = g
            lbaseA[t] = off
            g += KA[t]
            off += KA[t]
        for t in tiles:
            gbaseB[t] = g
            lbaseB[t] = off
            g += KB[t]
            off += KB[t]
        st_nchunks[s] = off
    TOTCH = g

    # gather calls: (supertile, half) -> num idxs + col offset in idx array
    call_n = np.zeros((NST, 2), dtype=np.int64)
    for s, tiles in enumerate(st_tiles):
        call_n[s, 0] = sum(KA[t] for t in tiles) * TILE
        call_n[s, 1] = sum(KB[t] for t in tiles) * TILE
    call_coloff = np.zeros((NST, 2), dtype=np.int64)
    col = 0
    for s in range(NST):
        for h in range(2):
            call_coloff[s, h] = col
            col += call_n[s, h] // 16
    TOTIDX16 = col  # idx array free-dim length (int16 columns)

    # slot of each (tile, half) group's first message within its gather call
    call_slot_base = np.zeros((NT, 2), dtype=np.int64)
    for s, tiles in enumerate(st_tiles):
        offA = 0
        offB = 0
        for t in tiles:
            call_slot_base[t, 0] = offA
            offA += KA[t] * TILE
            call_slot_base[t, 1] = offB
            offB += KB[t] * TILE

    # ---------------------------------------------------------------- per-msg
    # position within (core, tile, half) group
    gstart = np.zeros(NCORES * NT * 2 + 1, dtype=np.int64)
    np.cumsum(np.bincount(gid_s, minlength=NCORES * NT * 2), out=gstart[1:])
    pos = np.arange(M, dtype=np.int64) - gstart[gid_s]

    m_core = gid_s // (NT * 2)
    m_tile = (gid_s // 2) % NT
    m_half = gid_s % 2
    m_chunk_in_group = pos // TILE
    m_part = pos % TILE
    m_gchunk = np.where(m_half == 0, gbaseA[m_tile], gbaseB[m_tile]) + m_chunk_in_group
    m_dstloc = gdst - (m_core * NPC + m_tile * TILE)
    m_idx16 = np.where(m_half == 0, gsrc, gsrc - HALF).astype(np.int16)
    m_slot = (
        call_slot_base[m_tile, m_half] + pos
    )  # slot within the gather call
    m_col = call_coloff[st_of_tile[m_tile], m_half] + m_slot // 16
    m_row16 = m_slot % 16

    # ---------------------------------------------------------------- arrays
    plan = Plan()
    plan.N, plan.NPC, plan.NT, plan.NTP, plan.NST = N, NPC, NT, NTP, NST
    plan.st_tiles = st_tiles
    plan.KA, plan.KB, plan.K = KA, KB, K
    plan.gbaseA, plan.gbaseB = gbaseA, gbaseB
    plan.st_chunk_off, plan.st_nchunks = st_chunk_off, st_nchunks
    plan.call_n, plan.call_coloff = call_n, call_coloff
    plan.TOTCH, plan.TOTIDX16 = TOTCH, TOTIDX16
    plan.dinv = dinv

    per_core = []
    for c in range(NCORES):
        sel = m_core == c
        # S one-hot [128, TOTCH*128] fp8
        S = np.zeros((TILE, TOTCH * TILE), dtype=ml_dtypes.float8_e4m3)
        S[m_part[sel], m_gchunk[sel] * TILE + m_dstloc[sel]] = 1.0
        # idx [128, TOTIDX16] int16 (wrapped by 16, replicated across 8 groups)
        idx16 = np.zeros((16, TOTIDX16), dtype=np.int16)
        idx16[m_row16[sel], m_col[sel]] = m_idx16[sel]
        idx = np.tile(idx16, (8, 1))
        # xT [96, NTP] f32
        xT = np.zeros((D, NTP), dtype=np.float32)
        xT[:, :NPC] = x[c * NPC : (c + 1) * NPC].T
        # dinvT replicated [96, NTP]
        dinvT = np.ones((D, NTP), dtype=np.float32)
        dinvT[:, :NPC] = dinv[c * NPC : (c + 1) * NPC][None, :]
        # dinv per own row, tile-column layout [128, NT]
        downv = np.ones((TILE, NT), dtype=np.float32)
        dv = dinv[c * NPC : (c + 1) * NPC]
        dvp = np.zeros(NTP, dtype=np.float32)
        dvp[:NPC] = dv
        downv[:, :] = dvp.reshape(NT, TILE).T
        per_core.append(dict(S=S, idx=idx, xT=xT, dinvT=dinvT, dinvown=downv))
    plan.per_core = per_core
    return plan


# ---------------------------------------------------------------------------
# Bass program builder
# ---------------------------------------------------------------------------


def _build(plan, repeats=1, skip=frozenset()):
    import concourse.bass as bass
    import concourse.bacc as bacc
    import concourse.mybir as mybir
    import concourse.tile as tile

    f32 = mybir.dt.float32
    bf16 = mybir.dt.bfloat16
    fp8 = mybir.dt.float8e4
    i16 = mybir.dt.int16
    AF = mybir.ActivationFunctionType
    ALU = mybir.AluOpType

    N, NPC, NT, NTP, NST = plan.N, plan.NPC, plan.NT, plan.NTP, plan.NST
    TOTCH, TOTIDX16 = plan.TOTCH, plan.TOTIDX16
    KA, KB = plan.KA, plan.KB

    nc = bacc.Bacc(None, target_bir_lowering=False, num_swdge_queues=4)

    xT_p = nc.declare_dram_parameter("xT", [D, NTP], f32, isOutput=False)
    idx_p = nc.declare_dram_parameter("idx", [TILE, TOTIDX16], i16, isOutput=False)
    s_p = nc.declare_dram_parameter("S", [TILE, TOTCH * TILE], fp8, isOutput=False)
    dinvT_p = nc.declare_dram_parameter("dinvT", [D, NTP], f32, isOutput=False)
    dinvown_p = nc.declare_dram_parameter("dinvown", [TILE, NT], f32, isOutput=False)
    biasT_p = nc.declare_dram_parameter("biasT", [D, N_LAYERS], f32, isOutput=False)
    brep_p = nc.declare_dram_parameter("brep", [TILE, 4], f32, isOutput=False)
    w_p = nc.declare_dram_parameter("W", [D, N_LAYERS * D], f32, isOutput=False)
    wl_p = nc.declare_dram_parameter("Wl", [D, 4], f32, isOutput=False)
    out_p = nc.declare_dram_parameter("out", [NPC, 4], f32, isOutput=True)

    replica_groups = [list(range(NCORES))]

    with tile.TileContext(nc) as tc:
        with (
            tc.tile_pool(name="persist", bufs=1) as persist,
            tc.tile_pool(name="hrelu", bufs=2) as hrelu_pool,
            tc.tile_pool(name="msg", bufs=MSG_BUFS) as msg_pool,
            tc.tile_pool(name="spool", bufs=2) as s_pool,
            tc.tile_pool(name="tmp", bufs=3) as tmp_pool,
            tc.tile_pool(name="stage", bufs=3) as stage_pool,
            tc.tile_pool(name="small", bufs=3) as small_pool,
            tc.tile_pool(name="psA", bufs=6, space="PSUM") as psA_pool,
            tc.tile_pool(name="ps3", bufs=2, space="PSUM") as ps3_pool,
            tc.tile_pool(name="dram", bufs=2, space="DRAM") as dram_pool,
        ):
            # ------------------------------------------------- persistent loads
            xT_sb = persist.tile([D, NTP], bf16, tag="xT")
            nc.gpsimd.dma_start(xT_sb[:], xT_p[:])  # f32 -> bf16 cast DMA
            idx_sb = persist.tile([TILE, TOTIDX16], i16, tag="idx")
            nc.sync.dma_start(idx_sb[:], idx_p[:])
            dinvT_sb = persist.tile([D, NTP], f32, tag="dinvT")
            nc.sync.dma_start(dinvT_sb[:], dinvT_p[:])
            dinvown_sb = persist.tile([TILE, NT], f32, tag="dinvown")
            nc.sync.dma_start(dinvown_sb[:], dinvown_p[:])
            biasT_sb = persist.tile([D, N_LAYERS], f32, tag="biasT")
            nc.sync.dma_start(biasT_sb[:], biasT_p[:])
            brep_sb = persist.tile([TILE, 4], f32, tag="brep")
            nc.sync.dma_start(brep_sb[:], brep_p[:])
            w_sb = persist.tile([D, N_LAYERS * D], bf16, tag="W")
            nc.gpsimd.dma_start(w_sb[:], w_p[:])
            wl_sb = persist.tile([D, 4], bf16, tag="Wl")
            nc.gpsimd.dma_start(wl_sb[:], wl_p[:])

            cur_T = xT_sb  # [96, NTP] bf16, transposed feature table

            for rep in range(repeats):
              cur_T = xT_sb
              for layer in range(N_LAYERS):
                # ---------------------------------------- transform + stage out
                ag_in = dram_pool.tile([NPC, TILE], bf16, tag="ag_in")
                ag_out = dram_pool.tile(
                    [N, TILE],
                    bf16,
                    tag="ag_out",
                    addr_space="Local" if "ag" in skip else "Shared",
                )
                for t in range(NT):
                    w = min(TILE, NPC - t * TILE)
                    ps3 = ps3_pool.tile([TILE, D], f32, tag="ps3")
                    nc.tensor.matmul(
                        ps3[:w],
                        cur_T[:, t * TILE : t * TILE + w],
                        w_sb[:, layer * D : (layer + 1) * D],
                        start=True,
                        stop=True,
                    )
                    st = stage_pool.tile([TILE, TILE], bf16, tag="stage")
                    nc.vector.memset(st[:w, D:TILE], 0.0)
                    nc.scalar.activation(
                        st[:w, 0:D], ps3[:w], AF.Copy, scale=dinvown_sb[:w, t : t + 1]
                    )
                    nc.sync.dma_start(ag_in[t * TILE : t * TILE + w, :], st[:w])

                if "ag" in skip:
                    for r in range(NCORES):
                        nc.sync.dma_start(ag_out[r * NPC : (r + 1) * NPC, :], ag_in[:])
                else:
                    nc.gpsimd.collective_compute(
                        "AllGather",
                        ALU.bypass,
                        replica_groups=replica_groups,
                        ins=[ag_in[:]],
                        outs=[ag_out[:]],
                    )

                # ---------------------------------------- gather + aggregate
                new_T = hrelu_pool.tile([D, NTP], bf16, tag="hrelu")
                for s, tiles in enumerate(plan.st_tiles):
                    nch = int(plan.st_nchunks[s])
                    goff = int(plan.st_chunk_off[s])
                    msg = msg_pool.tile([TILE, nch * TILE], bf16, tag="msg")
                    msg3 = msg[:].rearrange("p (c e) -> p c e", e=TILE)
                    s_sb = s_pool.tile([TILE, nch * TILE], fp8, tag="spool")
                    g_live = "gather" not in skip
                    s_live = "sdma" not in skip
                    if s_live:
                        nc.sync.dma_start(
                            s_sb[:], s_p[:, goff * TILE : (goff + nch) * TILE]
                        )
                    else:
                        nc.sync.dma_start(
                            s_sb[:, 0:TILE], s_p[:, goff * TILE : (goff + 1) * TILE]
                        )
                    # gather calls (A half then B half), split to <=1024
                    # indices per call (SWDGE descriptor-ring capacity)
                    GMAX = 1024
                    ch_off = 0
                    gq = [0]
                    for h in range(2):
                        n = int(plan.call_n[s, h])
                        if n == 0:
                            continue
                        coloff = int(plan.call_coloff[s, h])
                        in_ap = ag_out[:] if h == 0 else ag_out[HALF:N, :]
                        if not g_live:
                            if h == 0:
                                nc.gpsimd.dma_gather(
                                    msg3[:, 0:1, :], in_ap,
                                    idx_sb[:, coloff : coloff + 8],
                                    num_idxs=TILE, num_idxs_reg=TILE, elem_size=TILE,
                                )
                            continue
                        for c0 in range(0, n, GMAX):
                            nn = min(GMAX, n - c0)
                            out_ap = msg3[
                                :, ch_off + c0 // TILE : ch_off + (c0 + nn) // TILE, :
                            ]
                            nc.gpsimd.dma_gather(
                                out_ap,
                                in_ap,
                                idx_sb[:, coloff + c0 // 16 : coloff + (c0 + nn) // 16],
                                num_idxs=nn,
                                num_idxs_reg=nn,
                                elem_size=TILE,
                                queue_num=gq[0] % 4,
                                single_packet=SINGLE_PACKET,
                            )
                            gq[0] += 1
                        ch_off += n // TILE
                    # per-tile accumulation, tile-major chunk order
                    for t in tiles:
                        kA, kB = int(KA[t]), int(KB[t])
                        nchunks_t = kA + kB
                        assert nchunks_t > 0
                        lA = int(plan.gbaseA[t] - goff)
                        lB = int(plan.gbaseB[t] - goff)
                        locs = [lA + j for j in range(kA)] + [lB + j for j in range(kB)]
                        psA = psA_pool.tile([D, TILE], f32, tag="psA")
                        for ji, j in enumerate(locs if "mm" not in skip else locs[:1]):
                            nc.tensor.matmul(
                                psA[:],
                                msg3[:, j if g_live else 0, 0:D],
                                s_sb[:, (j if s_live else 0) * TILE : ((j if s_live else 0) + 1) * TILE],
                                start=(ji == 0),
                                stop=(ji == nchunks_t - 1),
                            )
                        tmp = tmp_pool.tile([D, TILE], f32, tag="tmp")
                        nc.vector.tensor_tensor(
                            tmp[:], psA[:], dinvT_sb[:, t * TILE : (t + 1) * TILE], ALU.mult
                        )
                        nc.scalar.activation(
                            new_T[:, t * TILE : (t + 1) * TILE],
                            tmp[:],
                            AF.Relu,
                            bias=biasT_sb[:, layer : layer + 1],
                        )
                cur_T = new_T

            # ------------------------------------------------- classifier
            for t in range(NT):
                w = min(TILE, NPC - t * TILE)
                psf = ps3_pool.tile([TILE, D], f32, tag="ps3")
                nc.tensor.matmul(
                    psf[:w, 0:4],
                    cur_T[:, t * TILE : t * TILE + w],
                    wl_sb[:],
                    start=True,
                    stop=True,
                )
                xb = small_pool.tile([TILE, 4], f32, tag="xb")
                nc.vector.tensor_tensor(xb[:w], psf[:w, 0:4], brep_sb[:w], ALU.add)
                negm = small_pool.tile([TILE, 1], f32, tag="negm")
                nc.vector.tensor_reduce(
                    negm[:w], xb[:w], mybir.AxisListType.X, ALU.max, negate=True
                )
                ex = small_pool.tile([TILE, 4], f32, tag="ex")
                sumexp = small_pool.tile([TILE, 1], f32, tag="sumexp")
                nc.scalar.activation(
                    ex[:w], xb[:w], AF.Exp, bias=negm[:w], accum_out=sumexp[:w]
                )
                lse = small_pool.tile([TILE, 1], f32, tag="lse")
                nc.scalar.activation(lse[:w], sumexp[:w], AF.Ln)
                shift = small_pool.tile([TILE, 1], f32, tag="shift")
                nc.vector.tensor_sub(shift[:w], negm[:w], lse[:w])
                outt = small_pool.tile([TILE, 4], f32, tag="outt")
                nc.vector.tensor_scalar_add(outt[:w], xb[:w], shift[:w])
                nc.sync.dma_start(out_p[t * TILE : t * TILE + w, :], outt[:w])

    nc.compile()
    return nc


# ---------------------------------------------------------------------------
# in_maps assembly
# ---------------------------------------------------------------------------


def _in_maps(plan, W0, b0, W1, b1, W2, b2, W3, b3, Wl, bl):
    Ws = np.concatenate(
        [np.asarray(w, np.float32) for w in (W0, W1, W2, W3)], axis=1
    )  # [96, 4*96]
    biasT = np.stack(
        [np.asarray(b, np.float32) for b in (b0, b1, b2, b3)], axis=1
    )  # [96, 4]
    brep = np.tile(np.asarray(bl, np.float32)[None, :], (TILE, 1))  # [128, 4]
    wl = np.asarray(Wl, np.float32)
    maps = []
    for c in range(NCORES):
        pc = plan.per_core[c]
        maps.append(
            {
                "xT": pc["xT"],
                "idx": pc["idx"],
                "S": pc["S"],
                "dinvT": pc["dinvT"],
                "dinvown": pc["dinvown"],
                "biasT": biasT,
                "brep": brep,
                "W": Ws,
                "Wl": wl,
                "out": np.zeros((plan.NPC, 4), np.float32),
            }
        )
    return maps


# ---------------------------------------------------------------------------
# public entry point
# ---------------------------------------------------------------------------

_CACHE = {}


def _get_compiled(plan):
    return _build(plan)


def kernel(x, edge_index, W0, b0, W1, b1, W2, b2, W3, b3, Wl, bl):
    from concourse.bass_utils import run_bass_kernel_spmd

    x = np.asarray(x, np.float32)
    edge_index = np.asarray(edge_index, np.int64)
    plan = _prep(x, edge_index)
    nc = _get_compiled(plan)
    in_maps = _in_maps(plan, W0, b0, W1, b1, W2, b2, W3, b3, Wl, bl)
    res = run_bass_kernel_spmd(nc, in_maps, core_ids=list(range(NCORES)))
    out = np.concatenate([res.results[c]["out"] for c in range(NCORES)], axis=0)
    return out.astype(np.float32)
